# revision 20
# baseline (speedup 1.0000x reference)
"""GNN message-passing kernel for trn2: h = relu(BN(s1 @ W_pre));
agg = segment_sum(h[src], dst); out = relu((1-b)*support + b*support@W_op),
support = 0.9*(h+agg) + 0.1*x_0.

Sharding: phase 1 (h) replicated on all 8 cores; phase 2 (aggregate+output)
sharded by destination node. Gather via dma_gather on bf16 h tables (lo/hi
split for int16 indices). Segment-sum via selection-matrix matmuls.
"""
import math
import numpy as np
import ml_dtypes

import concourse.bass as bass
import concourse.bacc as bacc
import concourse.mybir as mybir
from concourse.tile import TileContext

BF16 = mybir.dt.bfloat16
F32 = mybir.dt.float32
I16 = mybir.dt.int16

ALPHA = 0.1
LAMBDA = 0.5
BN_EPS = 1e-5
BETA_C = float(np.log(LAMBDA / 1.0 + 1.0))   # 0.405465
W_OP_SCALE = BETA_C / (1.0 - BETA_C)         # fold: u = support + support@ (W_op*W_OP_SCALE)
OUT_SCALE = 1.0 - BETA_C                     # out = relu(OUT_SCALE * u)


class Prob:
    def __init__(self, N, E, C, HID, n_cores):
        self.N, self.E, self.C, self.HID, self.n_cores = N, E, C, HID, n_cores
        assert C == 256 and HID == 256
        self.shard = N // n_cores                      # dst nodes per core (must divide)
        assert self.shard * n_cores == N
        self.tiles = math.ceil(self.shard / 128)       # dst tiles per core
        self.shard_pad = self.tiles * 128
        self.nchunks = math.ceil(N / 128)              # node chunks for h
        self.npad = self.nchunks * 128
        self.lo_chunks = (self.nchunks + 1) // 2       # h_lo = chunks [0, lo_chunks)
        self.V_lo = self.lo_chunks * 128
        self.V_hi = self.npad - self.V_lo
        assert self.V_lo < 32768 and self.V_hi < 32768


def host_prep(prob, s1, x_0, edge_index):
    """Build per-core input maps + layout metadata. All numpy."""
    p = prob
    N, E, M = p.N, p.E, p.n_cores
    s1 = np.asarray(s1, dtype=np.float32)
    x_0 = np.asarray(x_0, dtype=np.float32)
    src = np.asarray(edge_index[0], dtype=np.int64)
    dst = np.asarray(edge_index[1], dtype=np.int64)
    # append self edges i->i (the GIN +h term)
    src = np.concatenate([src, np.arange(N, dtype=np.int64)])
    dst = np.concatenate([dst, np.arange(N, dtype=np.int64)])

    core = dst // p.shard
    rel = dst - core * p.shard
    trel = rel >> 7
    prel = rel & 127
    half = (src >= p.V_lo).astype(np.int64)
    gid = (core * p.tiles + trel) * 2 + half
    ngroups = M * p.tiles * 2
    order = np.argsort(gid, kind="stable")
    # counts per group
    cnt = np.bincount(gid, minlength=ngroups).reshape(M, p.tiles, 2)
    # chunks per (tile, half): max over cores (shared NEFF layout)
    K = np.maximum(np.ceil(cnt / 128.0).astype(np.int64).max(axis=0), 1)  # [tiles, 2]
    slots = K * 128
    # column offsets of each (t, half) group in the concatenated layout
    off = np.zeros((p.tiles, 2), np.int64)
    run = 0
    for t in range(p.tiles):
        for h in (0, 1):
            off[t, h] = run
            run += K[t, h]
    ktot = run                       # total chunks per core
    # build per-core padded idx/drel arrays
    src_s = src[order]
    drel_s = prel[order]
    gid_s = gid[order]
    # starts of each group in sorted arrays
    gstart = np.zeros(ngroups + 1, np.int64)
    np.cumsum(np.bincount(gid_s, minlength=ngroups), out=gstart[1:])
    idx_flat = np.zeros((M, ktot * 128), np.int16)
    drel_flat = np.full((M, ktot * 128), 200.0, np.float32)  # cast later
    for m in range(M):
        for t in range(p.tiles):
            for h in (0, 1):
                g = (m * p.tiles + t) * 2 + h
                a, b = gstart[g], gstart[g + 1]
                n = b - a
                base = off[t, h] * 128
                v = src_s[a:b] - (p.V_lo if h else 0)
                # chunk-major h table: node (j*128+p') lives at flat row p'*tchunks + j
                tch = p.lo_chunks if h == 0 else (p.nchunks - p.lo_chunks)
                v = (v & 127) * tch + (v >> 7)
                idx_flat[m, base:base + n] = v.astype(np.int16)
                drel_flat[m, base:base + n] = drel_s[a:b].astype(np.float32)
    # dma_gather idx layout: index j at [j%16, j//16], replicated x8 down partitions
    idx_lay = idx_flat.reshape(M, ktot * 8, 16).transpose(0, 2, 1)  # [M, 16, ktot*8]
    idx_lay = np.tile(idx_lay, (1, 8, 1))                           # [M, 128, ktot*8]
    # drel layout: [128, ktot]: drel[p, c] = flat[c*128 + p]
    drel_lay = np.ascontiguousarray(drel_flat.reshape(M, ktot, 128).transpose(0, 2, 1))
    # host-precomputed selection matrices: S_all[m, p, c*128+j] = (drel==j)
    dr = drel_flat.reshape(M, ktot, 128)            # [m, c, p]
    S_all = np.zeros((M, 128, ktot * 128), ml_dtypes.bfloat16)
    vm = dr < 128
    mi, ci, pi = np.nonzero(vm)
    ji = dr[vm].astype(np.int64)
    S_all[mi, pi, ci * 128 + ji] = 1.0

    # s1 extended: [npad, 257] bf16 (col 256 = 1.0 for real rows)
    s1e = np.zeros((p.npad, p.C + 1), np.float32)
    s1e[:N, :p.C] = s1
    s1e[:N, p.C] = 1.0
    # chunk-major relayout: [128, nchunks*(C+1)], partition p holds row j*128+p of chunk j
    s1e = s1e.reshape(p.nchunks, 128, p.C + 1).transpose(1, 0, 2).reshape(128, -1)
    s1e = np.ascontiguousarray(s1e).astype(ml_dtypes.bfloat16)
    # s1 transposed [C, npad] bf16
    s1T = np.zeros((p.C, p.npad), np.float32)
    s1T[:, :N] = s1.T
    s1T = np.ascontiguousarray(s1T).astype(ml_dtypes.bfloat16)

    x0s = np.zeros((M, p.shard_pad, p.HID), np.float32)
    for m in range(M):
        x0s[m, :p.shard] = x_0[m * p.shard:(m + 1) * p.shard]
    x0s = x0s.reshape(M, p.tiles, 128, p.HID).transpose(0, 2, 1, 3).reshape(M, 128, -1)
    x0s = np.ascontiguousarray(x0s)

    iota = np.broadcast_to(np.arange(128, dtype=np.float32), (128, 128)).astype(ml_dtypes.bfloat16).copy()
    ident = np.eye(128, dtype=np.float32)
    ones1 = np.ones((1, 128), np.float32)
    onesc = np.ones((128, 1), np.float32)

    meta = dict(K=K, off=off, ktot=ktot)
    shared = dict(s1e=np.asarray(s1e), s1T=np.asarray(s1T), iota=iota, ident=ident,
                  ones1=ones1, onesc=onesc)
    in_maps = []
    for m in range(M):
        d = dict(shared)
        d["idxall"] = idx_lay[m]
        d["drel"] = drel_lay[m]
        d["Sall"] = np.ascontiguousarray(S_all[m])
        d["x0s"] = x0s[m]
        in_maps.append(d)
    return in_maps, meta


def build_kernel(prob, meta, W_pre, gamma, beta_bn, W_op, nloop=1, nq=4, phases='ABC', c_parts='gse'):
    """Build + compile the Bacc kernel. Weights are compile-time-ish inputs
    (still passed as tensors; only meta layout is baked)."""
    p = prob
    K, off, ktot = meta["K"], meta["off"], meta["ktot"]
    C, HID = p.C, p.HID
    nc = bacc.Bacc("TRN2", target_bir_lowering=False, debug=False,
                   num_devices=p.n_cores, num_swdge_queues=nq)
    t_s1e = nc.dram_tensor("s1e", [128, p.nchunks * (C + 1)], BF16, kind="ExternalInput")
    t_s1T = nc.dram_tensor("s1T", [C, p.npad], BF16, kind="ExternalInput")
    t_wpre = nc.dram_tensor("wpre", [C, HID], F32, kind="ExternalInput")
    t_gamma = nc.dram_tensor("gamma", [1, HID], F32, kind="ExternalInput")
    t_beta = nc.dram_tensor("beta", [1, HID], F32, kind="ExternalInput")
    t_wop = nc.dram_tensor("wop", [HID, HID], F32, kind="ExternalInput")
    t_x0 = nc.dram_tensor("x0s", [128, p.tiles * HID], F32, kind="ExternalInput")
    t_idx = nc.dram_tensor("idxall", [128, ktot * 8], I16, kind="ExternalInput")
    t_drel = nc.dram_tensor("drel", [128, ktot], F32, kind="ExternalInput")
    t_S = nc.dram_tensor("Sall", [128, ktot * 128], BF16, kind="ExternalInput")
    t_iota = nc.dram_tensor("iota", [128, 128], BF16, kind="ExternalInput")
    t_ident = nc.dram_tensor("ident", [128, 128], F32, kind="ExternalInput")
    t_ones1 = nc.dram_tensor("ones1", [1, 128], F32, kind="ExternalInput")
    t_onesc = nc.dram_tensor("onesc", [128, 1], F32, kind="ExternalInput")
    t_out = nc.dram_tensor("out", [128, p.tiles * HID], F32, kind="ExternalOutput")
    lo_ch = p.lo_chunks
    hi_ch = p.nchunks - p.lo_chunks
    h_lo = nc.dram_tensor("h_lo", [128, lo_ch * HID], BF16)
    h_hi = nc.dram_tensor("h_hi", [128, hi_ch * HID], BF16)

    SPAN = 64  # chunks per s1T span load

    # ---------------- context 1: stats + h ----------------
    tc1 = TileContext(nc)
    with tc1 as tc:
        with (tc.tile_pool(name="const", bufs=1) as cpool,
              tc.tile_pool(name="s1in", bufs=4) as apool,
              tc.tile_pool(name="span", bufs=2) as spool,
              tc.tile_pool(name="hout", bufs=3) as hpool,
              tc.tile_pool(name="psA", bufs=1, space="PSUM") as psA,
              tc.tile_pool(name="psZ", bufs=5, space="PSUM") as psZ,
              tc.tile_pool(name="small", bufs=1) as smpool):
            # constants
            w_f32 = []
            for r in range(2):
                w = cpool.tile([128, HID], F32, tag=f"wf{r}")
                nc.sync.dma_start(out=w[:], in_=t_wpre[r * 128:(r + 1) * 128, :])
                w_f32.append(w)
            gamma_sb = cpool.tile([1, HID], F32, tag="gm")
            nc.sync.dma_start(out=gamma_sb[:], in_=t_gamma[:])
            beta_sb = cpool.tile([1, HID], F32, tag="bt")
            nc.sync.dma_start(out=beta_sb[:], in_=t_beta[:])
            ones1_sb = cpool.tile([1, 128], F32, tag="on")
            nc.sync.dma_start(out=ones1_sb[:], in_=t_ones1[:])
            onesc_sb = cpool.tile([128, 1], F32, tag="onc")
            nc.sync.dma_start(out=onesc_sb[:], in_=t_onesc[:])
            wop_f32 = []
            for r in range(2):
                w = cpool.tile([128, HID], F32, tag=f"wo{r}")
                nc.sync.dma_start(out=w[:], in_=t_wop[r * 128:(r + 1) * 128, :])
                wop_f32.append(w)

            # ---- phase A: Gram ----
            gps = [psA.tile([128, C + 1], F32, tag=f"g{r}", name=f"gps{r}") for r in range(2)]
            ASPAN = 32
            naspans = math.ceil(p.nchunks / ASPAN)
            CW = C + 1
            def phaseA():
                for s in range(naspans):
                    j0 = s * ASPAN
                    j1 = min(p.nchunks, j0 + ASPAN)
                    s1t = apool.tile([128, ASPAN * CW], BF16, name="s1span")
                    nc.sync.dma_start(out=s1t[:, :(j1 - j0) * CW],
                                      in_=t_s1e[:, j0 * CW:j1 * CW])
                    for j in range(j0, j1):
                        co = (j - j0) * CW
                        for r in range(2):
                            nc.tensor.matmul(gps[r][:],
                                             lhsT=s1t[:, co + r * 128:co + r * 128 + 128],
                                             rhs=s1t[:, co:co + CW], start=(j == 0),
                                             stop=(j == p.nchunks - 1))
            if 'A' in phases:
                if nloop > 1:
                    with tc.For_i(0, nloop, 1):
                        phaseA()
                else:
                    phaseA()
            else:
                phaseA()  # once (stats needed)

            # ---- stats finalize (tiny, once) ----
            g_sb = []
            for r in range(2):
                g = smpool.tile([128, C + 1], F32, tag=f"gsb{r}")
                nc.vector.tensor_copy(out=g[:], in_=gps[r][:])
                g_sb.append(g)
            # B_mat = G @ W  (G symmetric; lhsT = G rows as [K,M])
            psB = psZ.tile([128, HID], F32, tag="zb")
            b_sb = []
            for r in range(2):
                for k in range(2):
                    nc.tensor.matmul(psB[:], lhsT=g_sb[k][:, r * 128:(r + 1) * 128],
                                     rhs=w_f32[k][:], start=(k == 0), stop=(k == 1))
                b = smpool.tile([128, HID], F32, tag=f"bsb{r}")
                nc.vector.tensor_copy(out=b[:], in_=psB[:])
                b_sb.append(b)

            def psum_colsum(tiles_in, tag):
                # sum over partitions of (tiles_in[0]+tiles_in[1]) -> [1, HID] in SBUF
                acc = smpool.tile([128, HID], F32, tag=tag)
                nc.vector.tensor_tensor(out=acc[:], in0=tiles_in[0][:],
                                        in1=tiles_in[1][:], op=mybir.AluOpType.add)
                ps = psA.tile([1, HID], F32, tag="zred", name=f"ps_{tag}")
                nc.tensor.matmul(ps[:], lhsT=onesc_sb[:], rhs=acc[:],
                                 start=True, stop=True)
                res = smpool.tile([1, HID], F32, tag=tag + "r", name=f"res_{tag}")
                nc.vector.tensor_copy(out=res[:], in_=ps[:])
                return res

            # mu_raw = sum_k s_k W[k, :]
            sw = []
            for r in range(2):
                t = smpool.tile([128, HID], F32, tag=f"sw{r}")
                nc.vector.tensor_scalar(out=t[:], in0=w_f32[r][:],
                                        scalar1=g_sb[r][:, C:C + 1], scalar2=None,
                                        op0=mybir.AluOpType.mult)
                sw.append(t)
            mu_acc = psum_colsum(sw, "mua")
            # d_raw = sum_k W[k,c] B[k,c]
            wb = []
            for r in range(2):
                t = smpool.tile([128, HID], F32, tag=f"wb{r}")
                nc.vector.tensor_tensor(out=t[:], in0=w_f32[r][:], in1=b_sb[r][:],
                                        op=mybir.AluOpType.mult)
                wb.append(t)
            d_acc = psum_colsum(wb, "da")
            invn = 1.0 / p.N
            mu = smpool.tile([1, HID], F32, tag="mu")
            nc.vector.tensor_scalar(out=mu[:], in0=mu_acc[:], scalar1=invn,
                                    scalar2=None, op0=mybir.AluOpType.mult)
            var = smpool.tile([1, HID], F32, tag="var")
            # var = d/N - mu^2
            nc.vector.tensor_scalar(out=var[:], in0=d_acc[:], scalar1=invn,
                                    scalar2=None, op0=mybir.AluOpType.mult)
            musq = smpool.tile([1, HID], F32, tag="musq")
            nc.vector.tensor_tensor(out=musq[:], in0=mu[:], in1=mu[:],
                                    op=mybir.AluOpType.mult)
            nc.vector.tensor_tensor(out=var[:], in0=var[:], in1=musq[:],
                                    op=mybir.AluOpType.subtract)
            nc.vector.tensor_scalar(out=var[:], in0=var[:], scalar1=BN_EPS,
                                    scalar2=None, op0=mybir.AluOpType.add)
            sq = smpool.tile([1, HID], F32, tag="sq")
            nc.scalar.activation(out=sq[:], in_=var[:],
                                 func=mybir.ActivationFunctionType.Sqrt,
                                 bias=0.0, scale=1.0)
            rs = smpool.tile([1, HID], F32, tag="rs")
            nc.vector.reciprocal(out=rs[:], in_=sq[:])
            a_vec = smpool.tile([1, HID], F32, tag="av")
            nc.vector.tensor_tensor(out=a_vec[:], in0=rs[:], in1=gamma_sb[:],
                                    op=mybir.AluOpType.mult)
            b_vec = smpool.tile([1, HID], F32, tag="bv")
            nc.vector.tensor_tensor(out=b_vec[:], in0=mu[:], in1=a_vec[:],
                                    op=mybir.AluOpType.mult)
            nc.vector.tensor_tensor(out=b_vec[:], in0=beta_sb[:], in1=b_vec[:],
                                    op=mybir.AluOpType.subtract)
            # broadcast A|B to 128 partitions via K=1 matmul
            ab_cat = smpool.tile([1, 2 * HID], F32, tag="abc")
            nc.vector.tensor_copy(out=ab_cat[:, :HID], in_=a_vec[:])
            nc.vector.tensor_copy(out=ab_cat[:, HID:], in_=b_vec[:])
            ps_ab = psZ.tile([128, 2 * HID], F32, tag="zb")
            nc.tensor.matmul(ps_ab[:], lhsT=ones1_sb[:], rhs=ab_cat[:],
                             start=True, stop=True)
            b_bc = cpool.tile([128, HID], F32, tag="bbc")
            nc.vector.tensor_copy(out=b_bc[:], in_=ps_ab[:, HID:])
            # scaled weights: Wsc = W * A (bf16); B row for K=1 add; Wop scaled
            wsc = []
            for r in range(2):
                w = cpool.tile([128, HID], BF16, tag=f"wsc{r}")
                nc.vector.tensor_tensor(out=w[:], in0=w_f32[r][:], in1=ps_ab[:, :HID],
                                        op=mybir.AluOpType.mult)
                wsc.append(w)
            b_row = cpool.tile([1, HID], BF16, tag="brow")
            nc.vector.tensor_copy(out=b_row[:], in_=b_vec[:])
            ones1_bf = cpool.tile([1, 128], BF16, tag="on16")
            nc.vector.tensor_copy(out=ones1_bf[:], in_=ones1_sb[:])
            wopsc = []
            for r in range(2):
                w = cpool.tile([128, HID], BF16, tag=f"wosc{r}")
                nc.vector.tensor_scalar(out=w[:], in0=wop_f32[r][:],
                                        scalar1=W_OP_SCALE, scalar2=None,
                                        op0=mybir.AluOpType.mult)
                wopsc.append(w)

            # ---- phase B: z = s1 @ Wsc (+B) -> relu -> h ----
            nspans = math.ceil(p.nchunks / SPAN)
            def phaseB():
                for s in range(nspans):
                    j0 = s * SPAN
                    j1 = min(p.nchunks, j0 + SPAN)
                    w_nodes = (j1 - j0) * 128
                    spans = []
                    for r in range(2):
                        sp = spool.tile([128, SPAN * 128], BF16, tag=f"sp{r}")
                        nc.sync.dma_start(
                            out=sp[:, :w_nodes],
                            in_=t_s1T[r * 128:(r + 1) * 128, j0 * 128:j0 * 128 + w_nodes])
                        spans.append(sp)
                    for j in range(j0, j1):
                        zc = psZ.tile([128, HID], F32, tag="zb")
                        coff = (j - j0) * 128
                        nc.tensor.matmul(zc[:], lhsT=spans[0][:, coff:coff + 128],
                                         rhs=wsc[0][:], start=True, stop=False)
                        nc.tensor.matmul(zc[:], lhsT=spans[1][:, coff:coff + 128],
                                         rhs=wsc[1][:], start=False, stop=True)
                        # h span buffering (chunk-major tables, 14-chunk spans)
                        HSPAN = 14
                        if j < p.lo_chunks:
                            tbl, jj, nch = h_lo, j, lo_ch
                        else:
                            tbl, jj, nch = h_hi, j - p.lo_chunks, hi_ch
                        hs = jj // HSPAN
                        ho = jj % HSPAN
                        he = min(nch, (hs + 1) * HSPAN) - hs * HSPAN
                        if ho == 0:
                            hsp = hpool.tile([128, HSPAN * HID], BF16, tag="hsp",
                                             name=f"hsp_{0 if tbl is h_lo else 1}_{hs % 3}")
                            phaseB.hsp = hsp
                        hsp = phaseB.hsp
                        zb = hpool.tile([128, HID], F32, tag="zb2")
                        nc.vector.tensor_tensor(out=zb[:], in0=zc[:], in1=b_bc[:],
                                                op=mybir.AluOpType.add)
                        if j % 3 == 0:
                            nc.vector.tensor_scalar(
                                out=hsp[:, ho * HID:(ho + 1) * HID], in0=zb[:],
                                scalar1=0.0, scalar2=None, op0=mybir.AluOpType.max)
                        elif j % 3 == 1:
                            nc.scalar.activation(
                                out=hsp[:, ho * HID:(ho + 1) * HID], in_=zb[:],
                                func=mybir.ActivationFunctionType.Relu,
                                bias=0.0, scale=1.0)
                        else:
                            nc.gpsimd.tensor_scalar(
                                out=hsp[:, ho * HID:(ho + 1) * HID], in0=zb[:],
                                scalar1=0.0, scalar2=None, op0=mybir.AluOpType.max)
                        if ho == he - 1:
                            nc.sync.dma_start(
                                out=tbl[:, hs * HSPAN * HID:(hs * HSPAN + he) * HID],
                                in_=hsp[:, :he * HID])
            if 'B' in phases:
                if nloop > 1:
                    with tc.For_i(0, nloop, 1):
                        phaseB()
                else:
                    phaseB()
            else:
                phaseB()  # once

        # ---------------- context 2: aggregate + output ----------------
        with (tc.tile_pool(name="c2", bufs=1) as cpool,
              tc.tile_pool(name="gat", bufs=10) as gpool,
              tc.tile_pool(name="sel", bufs=4) as selp,
              tc.tile_pool(name="epi", bufs=3) as epool,
              tc.tile_pool(name="psG", bufs=4, space="PSUM") as psG,
              tc.tile_pool(name="psT", bufs=2, space="PSUM") as psT,
              tc.tile_pool(name="psO", bufs=2, space="PSUM") as psO):
            idx_sb = cpool.tile([128, ktot * 8], I16, tag="idx")
            nc.sync.dma_start(out=idx_sb[:], in_=t_idx[:])
            drel_sb = cpool.tile([128, ktot], F32, tag="dr")
            nc.sync.dma_start(out=drel_sb[:], in_=t_drel[:])
            iota_sb = cpool.tile([128, 128], BF16, tag="io")
            nc.sync.dma_start(out=iota_sb[:], in_=t_iota[:])
            ident_sb = cpool.tile([128, 128], F32, tag="idn")
            nc.sync.dma_start(out=ident_sb[:], in_=t_ident[:])
            wop2 = []
            for r in range(2):
                w = cpool.tile([128, HID], F32, tag=f"wo2{r}")
                nc.sync.dma_start(out=w[:], in_=t_wop[r * 128:(r + 1) * 128, :])
                wb = cpool.tile([128, HID], BF16, tag=f"wo2b{r}")
                nc.vector.tensor_scalar(out=wb[:], in0=w[:], scalar1=W_OP_SCALE,
                                        scalar2=None, op0=mybir.AluOpType.mult)
                wop2.append(wb)

            qn = [0]
            def phaseC():
                for t in range(p.tiles):
                    gt = {}
                    st = {}
                    for hh in (0, 1) if 'g' in c_parts else ():
                        kk = int(K[t, hh])
                        g = gpool.tile([128, kk * HID], BF16, tag=f"g{hh}")
                        tbl = h_lo if hh == 0 else h_hi
                        o8 = int(off[t, hh]) * 8
                        nc.gpsimd.dma_gather(
                            out_ap=g[:].rearrange("p (c d) -> p c d", d=HID),
                            in_ap=tbl[:].rearrange("p (c d) -> (p c) d", d=HID),
                            idxs_ap=idx_sb[:, o8:o8 + kk * 8],
                            num_idxs=kk * 128, num_idxs_reg=kk * 128,
                            elem_size=HID, single_packet=False,
                            queue_num=qn[0] % 4)
                        qn[0] += 1
                        gt[hh] = g
                        ssp = selp.tile([128, kk * 128], BF16, tag=f"s{hh}")
                        o128 = int(off[t, hh]) * 128
                        nc.sync.dma_start(out=ssp[:], in_=t_S[:, o128:o128 + kk * 128])
                        st[hh] = ssp
                    agg = psG.tile([128, HID], F32, tag="agg")
                    nmm = int(K[t, 0] + K[t, 1])
                    ci = 0
                    if 's' not in c_parts or 'g' not in c_parts:
                        nc.vector.memset(agg[:], 0.0)
                    for hh in ((0, 1) if ('s' in c_parts and 'g' in c_parts) else ()):
                        kk = int(K[t, hh])
                        for c in range(kk):
                            nc.tensor.matmul(agg[:], lhsT=st[hh][:, c * 128:(c + 1) * 128],
                                             rhs=gt[hh][:, c * HID:(c + 1) * HID],
                                             start=(ci == 0), stop=(ci == nmm - 1))
                            ci += 1
                    # epilogue
                    if 'e' not in c_parts:
                        continue
                    ESPAN = 7
                    es = t // ESPAN
                    eo = t % ESPAN
                    e0 = es * ESPAN
                    e1 = min(p.tiles, e0 + ESPAN)
                    if eo == 0:
                        x0sp = epool.tile([128, ESPAN * HID], F32, tag="x0sp",
                                          name=f"x0sp{es % 2}")
                        nc.sync.dma_start(out=x0sp[:, :(e1 - e0) * HID],
                                          in_=t_x0[:, e0 * HID:e1 * HID])
                        outsp = epool.tile([128, ESPAN * HID], F32, tag="outsp",
                                           name=f"outsp{es % 2}")
                        phaseC.x0sp, phaseC.outsp = x0sp, outsp
                    x0sp, outsp = phaseC.x0sp, phaseC.outsp
                    sup = epool.tile([128, HID], F32, tag="sup")
                    # sup = 0.9*agg + 0.1*x0
                    nc.vector.tensor_scalar(out=sup[:], in0=agg[:],
                                            scalar1=(1.0 - ALPHA), scalar2=None,
                                            op0=mybir.AluOpType.mult)
                    x0sc = epool.tile([128, HID], F32, tag="x0sc")
                    nc.any.tensor_scalar(out=x0sc[:], in0=x0sp[:, eo * HID:(eo + 1) * HID],
                                         scalar1=ALPHA, scalar2=None,
                                         op0=mybir.AluOpType.mult)
                    nc.vector.tensor_tensor(out=sup[:], in0=sup[:], in1=x0sc[:],
                                            op=mybir.AluOpType.add)
                    # transpose sup -> supT (bf16)
                    trp = psT.tile([128, HID], F32, tag="tr")
                    for r in range(2):
                        nc.tensor.transpose(out=trp[:, r * 128:(r + 1) * 128],
                                            in_=sup[:, r * 128:(r + 1) * 128],
                                            identity=ident_sb[:])
                    supT = epool.tile([128, HID], BF16, tag="supT")
                    nc.any.tensor_copy(out=supT[:], in_=trp[:])
                    ops = psO.tile([128, HID], F32, tag="o")
                    nc.tensor.matmul(ops[:], lhsT=supT[:, :128], rhs=wop2[0][:],
                                     start=True, stop=False)
                    nc.tensor.matmul(ops[:], lhsT=supT[:, 128:], rhs=wop2[1][:],
                                     start=False, stop=True)
                    ut = epool.tile([128, HID], F32, tag="u")
                    nc.vector.tensor_tensor(out=ut[:], in0=sup[:], in1=ops[:],
                                            op=mybir.AluOpType.add)
                    nc.scalar.activation(out=outsp[:, eo * HID:(eo + 1) * HID],
                                         in_=ut[:],
                                         func=mybir.ActivationFunctionType.Relu,
                                         bias=0.0, scale=OUT_SCALE)
                    if t == e1 - 1:
                        nc.sync.dma_start(out=t_out[:, e0 * HID:e1 * HID],
                                          in_=outsp[:, :(e1 - e0) * HID])
            if 'C' in phases:
                if nloop > 1:
                    with tc.For_i(0, nloop, 1):
                        phaseC()
                else:
                    phaseC()

    nc.compile()
    return nc


def make_weight_inputs(prob, W_pre, gamma, beta_bn, W_op):
    return dict(
        wpre=np.asarray(W_pre, np.float32),
        gamma=np.asarray(gamma, np.float32).reshape(1, -1),
        beta=np.asarray(beta_bn, np.float32).reshape(1, -1),
        wop=np.asarray(W_op, np.float32),
    )


def unpack_out(prob, arr):
    """[128, tiles*HID] chunk-major -> [shard_pad, HID]"""
    return arr.reshape(128, prob.tiles, prob.HID).transpose(1, 0, 2).reshape(
        prob.shard_pad, prob.HID)


# ======================================================================
# Self-contained execution via PJRT (axon) and public kernel() entry
# ======================================================================
import jax
from jax.sharding import Mesh, PartitionSpec, NamedSharding
from jax.experimental.shard_map import shard_map
from concourse.bass2jax import _bass_exec_p, install_neuronx_cc_hook, partition_id_tensor


def _build_exec(nc, n_cores):
    install_neuronx_cc_hook()
    partition_name = nc.partition_id_tensor.name if nc.partition_id_tensor else None
    in_names, out_names, out_avals, zero_outs = [], [], [], []
    for alloc in nc.m.functions[0].allocations:
        if not isinstance(alloc, mybir.MemoryLocationSet):
            continue
        name = alloc.memorylocations[0].name
        if alloc.kind == "ExternalInput":
            if name != partition_name:
                in_names.append(name)
        elif alloc.kind == "ExternalOutput":
            shape = tuple(alloc.tensor_shape)
            dtype = mybir.dt.np(alloc.dtype)
            out_names.append(name)
            out_avals.append(jax.core.ShapedArray(shape, dtype))
            zero_outs.append(np.zeros(shape, dtype))
    n_params = len(in_names)
    n_outs = len(out_avals)
    all_in_names = list(in_names) + list(out_names)
    if partition_name is not None:
        all_in_names.append(partition_name)

    def _body(*args):
        operands = list(args)
        if partition_name is not None:
            operands.append(partition_id_tensor())
        outs = _bass_exec_p.bind(
            *operands, out_avals=tuple(out_avals), in_names=tuple(all_in_names),
            out_names=tuple(out_names), lowering_input_output_aliases=(),
            sim_require_finite=True, sim_require_nnan=True, nc=nc)
        return tuple(outs)

    devices = jax.devices()[:n_cores]
    mesh = Mesh(np.asarray(devices), ("core",))
    in_specs = (PartitionSpec("core"),) * (n_params + n_outs)
    out_specs = (PartitionSpec("core"),) * n_outs
    donate = tuple(range(n_params, n_params + n_outs))
    fn = jax.jit(shard_map(_body, mesh=mesh, in_specs=in_specs,
                           out_specs=out_specs, check_rep=False),
                 donate_argnums=donate, keep_unused=True)
    return dict(fn=fn, in_names=in_names, out_names=out_names,
                out_avals=out_avals, zero_outs=zero_outs, mesh=mesh,
                n_cores=n_cores)


def _place_inputs(ex, in_maps):
    sh = NamedSharding(ex["mesh"], PartitionSpec("core"))
    n_cores = ex["n_cores"]
    return [jax.device_put(
        np.concatenate([np.asarray(in_maps[c][name]) for c in range(n_cores)], axis=0), sh)
        for name in ex["in_names"]]


def _run(ex, dev_in):
    sh = NamedSharding(ex["mesh"], PartitionSpec("core"))
    n_cores = ex["n_cores"]
    zs = [jax.device_put(np.zeros((n_cores * z.shape[0], *z.shape[1:]), z.dtype), sh)
          for z in ex["zero_outs"]]
    outs = jax.block_until_ready(ex["fn"](*dev_in, *zs))
    return [
        {name: np.asarray(outs[i]).reshape(n_cores, *ex["out_avals"][i].shape)[c]
         for i, name in enumerate(ex["out_names"])}
        for c in range(n_cores)
    ]


_CACHE = {}


def _get_compiled(prob, meta, W_pre, gamma, beta_bn, W_op, key):
    if key not in _CACHE:
        nc = build_kernel(prob, meta, W_pre, gamma, beta_bn, W_op, nloop=1)
        _CACHE[key] = _build_exec(nc, prob.n_cores)
    return _CACHE[key]


def kernel(s0=None, s1=None, x_0=None, W_pre=None, gamma=None, beta_bn=None,
           W_op=None, edge_index=None, drop_prob=None, training=None, **_ignored):
    s1 = np.asarray(s1, np.float32)
    x_0 = np.asarray(x_0, np.float32)
    W_pre = np.asarray(W_pre, np.float32)
    gamma = np.asarray(gamma, np.float32)
    beta_bn = np.asarray(beta_bn, np.float32)
    W_op = np.asarray(W_op, np.float32)
    edge_index = np.asarray(edge_index)
    N, C = s1.shape
    HID = W_pre.shape[1]
    E = edge_index.shape[1]
    prob = Prob(N, E, C, HID, n_cores=8)
    in_maps, meta = host_prep(prob, s1, x_0, edge_index)
    key = (N, E, C, HID, int(np.int64(edge_index[:, ::97]).sum()), meta["ktot"])
    ex = _get_compiled(prob, meta, W_pre, gamma, beta_bn, W_op, key)
    wins = make_weight_inputs(prob, W_pre, gamma, beta_bn, W_op)
    full_maps = [{**m, **wins} for m in in_maps]
    dev_in = _place_inputs(ex, full_maps)
    res = _run(ex, dev_in)
    out = np.concatenate(
        [unpack_out(prob, res[m]["out"])[:prob.shard] for m in range(prob.n_cores)],
        axis=0)
    return np.ascontiguousarray(out[:N]).astype(np.float32)



# revision 21
# speedup vs baseline: 1.5555x; 1.5555x over previous
"""GNN message-passing kernel for trn2: h = relu(BN(s1 @ W_pre));
agg = segment_sum(h[src], dst); out = relu((1-b)*support + b*support@W_op),
support = 0.9*(h+agg) + 0.1*x_0.

Sharding: phase 1 (h) replicated on all 8 cores; phase 2 (aggregate+output)
sharded by destination node. Gather via dma_gather on bf16 h tables (lo/hi
split for int16 indices). Segment-sum via selection-matrix matmuls.
"""
import math
import numpy as np
import ml_dtypes

import concourse.bass as bass
import concourse.bacc as bacc
import concourse.mybir as mybir
from concourse.tile import TileContext

BF16 = mybir.dt.bfloat16
F32 = mybir.dt.float32
I16 = mybir.dt.int16

ALPHA = 0.1
LAMBDA = 0.5
BN_EPS = 1e-5
BETA_C = float(np.log(LAMBDA / 1.0 + 1.0))   # 0.405465
W_OP_SCALE = BETA_C / (1.0 - BETA_C)         # fold: u = support + support@ (W_op*W_OP_SCALE)
OUT_SCALE = 1.0 - BETA_C                     # out = relu(OUT_SCALE * u)


class Prob:
    def __init__(self, N, E, C, HID, n_cores):
        self.N, self.E, self.C, self.HID, self.n_cores = N, E, C, HID, n_cores
        assert C == 256 and HID == 256
        self.shard = N // n_cores                      # dst nodes per core (must divide)
        assert self.shard * n_cores == N
        self.tiles = math.ceil(self.shard / 128)       # dst tiles per core
        self.shard_pad = self.tiles * 128
        self.nchunks = math.ceil(N / 128)              # node chunks for h
        self.npad = self.nchunks * 128
        self.lo_chunks = (self.nchunks + 1) // 2       # h_lo = chunks [0, lo_chunks)
        self.V_lo = self.lo_chunks * 128
        self.V_hi = self.npad - self.V_lo
        assert self.V_lo < 32768 and self.V_hi < 32768


def host_prep(prob, s1, x_0, edge_index):
    """Build per-core input maps + layout metadata. All numpy."""
    p = prob
    N, E, M = p.N, p.E, p.n_cores
    s1 = np.asarray(s1, dtype=np.float32)
    x_0 = np.asarray(x_0, dtype=np.float32)
    src = np.asarray(edge_index[0], dtype=np.int64)
    dst = np.asarray(edge_index[1], dtype=np.int64)
    # append self edges i->i (the GIN +h term)
    src = np.concatenate([src, np.arange(N, dtype=np.int64)])
    dst = np.concatenate([dst, np.arange(N, dtype=np.int64)])

    core = dst // p.shard
    rel = dst - core * p.shard
    trel = rel >> 7
    prel = rel & 127
    half = (src >= p.V_lo).astype(np.int64)
    gid = (core * p.tiles + trel) * 2 + half
    ngroups = M * p.tiles * 2
    order = np.argsort(gid, kind="stable")
    # counts per group
    cnt = np.bincount(gid, minlength=ngroups).reshape(M, p.tiles, 2)
    # chunks per (tile, half): max over cores (shared NEFF layout)
    K = np.maximum(np.ceil(cnt / 128.0).astype(np.int64).max(axis=0), 1)  # [tiles, 2]
    slots = K * 128
    # column offsets of each (t, half) group in the concatenated layout
    off = np.zeros((p.tiles, 2), np.int64)
    run = 0
    for t in range(p.tiles):
        for h in (0, 1):
            off[t, h] = run
            run += K[t, h]
    ktot = run                       # total chunks per core
    # build per-core padded idx/drel arrays
    src_s = src[order]
    drel_s = prel[order]
    gid_s = gid[order]
    # starts of each group in sorted arrays
    gstart = np.zeros(ngroups + 1, np.int64)
    np.cumsum(np.bincount(gid_s, minlength=ngroups), out=gstart[1:])
    idx_flat = np.zeros((M, ktot * 128), np.int16)
    drel_flat = np.full((M, ktot * 128), 200.0, np.float32)  # cast later
    for m in range(M):
        for t in range(p.tiles):
            for h in (0, 1):
                g = (m * p.tiles + t) * 2 + h
                a, b = gstart[g], gstart[g + 1]
                n = b - a
                base = off[t, h] * 128
                v = src_s[a:b] - (p.V_lo if h else 0)
                # chunk-major h table: node (j*128+p') lives at flat row p'*tchunks + j
                tch = p.lo_chunks if h == 0 else (p.nchunks - p.lo_chunks)
                v = (v & 127) * tch + (v >> 7)
                idx_flat[m, base:base + n] = v.astype(np.int16)
                drel_flat[m, base:base + n] = drel_s[a:b].astype(np.float32)
    # dma_gather idx layout: index j at [j%16, j//16], replicated x8 down partitions
    idx_lay = idx_flat.reshape(M, ktot * 8, 16).transpose(0, 2, 1)  # [M, 16, ktot*8]
    idx_lay = np.tile(idx_lay, (1, 8, 1))                           # [M, 128, ktot*8]
    # drel layout: [128, ktot]: drel[p, c] = flat[c*128 + p]
    drel_lay = np.ascontiguousarray(drel_flat.reshape(M, ktot, 128).transpose(0, 2, 1))
    # host-precomputed selection matrices: S_all[m, p, c*128+j] = (drel==j)
    dr = drel_flat.reshape(M, ktot, 128)            # [m, c, p]
    S_all = np.zeros((M, 128, ktot * 128), ml_dtypes.bfloat16)
    vm = dr < 128
    mi, ci, pi = np.nonzero(vm)
    ji = dr[vm].astype(np.int64)
    S_all[mi, pi, ci * 128 + ji] = 1.0

    # s1 extended: [npad, 257] bf16 (col 256 = 1.0 for real rows)
    s1e = np.zeros((p.npad, p.C + 1), np.float32)
    s1e[:N, :p.C] = s1
    s1e[:N, p.C] = 1.0
    # chunk-major relayout: [128, nchunks*(C+1)], partition p holds row j*128+p of chunk j
    s1e = s1e.reshape(p.nchunks, 128, p.C + 1).transpose(1, 0, 2).reshape(128, -1)
    s1e = np.ascontiguousarray(s1e).astype(ml_dtypes.bfloat16)
    # s1 transposed [C, npad] bf16
    s1T = np.zeros((p.C, p.npad), np.float32)
    s1T[:, :N] = s1.T
    s1T = np.ascontiguousarray(s1T).astype(ml_dtypes.bfloat16)

    x0s = np.zeros((M, p.shard_pad, p.HID), np.float32)
    for m in range(M):
        x0s[m, :p.shard] = x_0[m * p.shard:(m + 1) * p.shard]
    x0s = x0s.reshape(M, p.tiles, 128, p.HID).transpose(0, 2, 1, 3).reshape(M, 128, -1)
    x0s = np.ascontiguousarray(x0s)

    iota = np.broadcast_to(np.arange(128, dtype=np.float32), (128, 128)).astype(ml_dtypes.bfloat16).copy()
    ident = np.eye(128, dtype=np.float32)
    ones1 = np.ones((1, 128), np.float32)
    onesc = np.ones((128, 1), np.float32)

    meta = dict(K=K, off=off, ktot=ktot)
    shared = dict(s1e=np.asarray(s1e), s1T=np.asarray(s1T), iota=iota, ident=ident,
                  ones1=ones1, onesc=onesc)
    in_maps = []
    for m in range(M):
        d = dict(shared)
        d["idxall"] = idx_lay[m]
        d["drel"] = drel_lay[m]
        d["Sall"] = np.ascontiguousarray(S_all[m])
        d["x0s"] = x0s[m]
        in_maps.append(d)
    return in_maps, meta


def build_kernel(prob, meta, W_pre, gamma, beta_bn, W_op, nloop=1, nq=4, phases='ABC', c_parts='gse'):
    """Build + compile the Bacc kernel. Weights are compile-time-ish inputs
    (still passed as tensors; only meta layout is baked)."""
    p = prob
    K, off, ktot = meta["K"], meta["off"], meta["ktot"]
    C, HID = p.C, p.HID
    nc = bacc.Bacc("TRN2", target_bir_lowering=False, debug=False,
                   num_devices=p.n_cores, num_swdge_queues=nq)
    t_s1e = nc.dram_tensor("s1e", [128, p.nchunks * (C + 1)], BF16, kind="ExternalInput")
    t_s1T = nc.dram_tensor("s1T", [C, p.npad], BF16, kind="ExternalInput")
    t_wpre = nc.dram_tensor("wpre", [C, HID], F32, kind="ExternalInput")
    t_gamma = nc.dram_tensor("gamma", [1, HID], F32, kind="ExternalInput")
    t_beta = nc.dram_tensor("beta", [1, HID], F32, kind="ExternalInput")
    t_wop = nc.dram_tensor("wop", [HID, HID], F32, kind="ExternalInput")
    t_x0 = nc.dram_tensor("x0s", [128, p.tiles * HID], F32, kind="ExternalInput")
    t_idx = nc.dram_tensor("idxall", [128, ktot * 8], I16, kind="ExternalInput")
    t_drel = nc.dram_tensor("drel", [128, ktot], F32, kind="ExternalInput")
    t_S = nc.dram_tensor("Sall", [128, ktot * 128], BF16, kind="ExternalInput")
    t_iota = nc.dram_tensor("iota", [128, 128], BF16, kind="ExternalInput")
    t_ident = nc.dram_tensor("ident", [128, 128], F32, kind="ExternalInput")
    t_ones1 = nc.dram_tensor("ones1", [1, 128], F32, kind="ExternalInput")
    t_onesc = nc.dram_tensor("onesc", [128, 1], F32, kind="ExternalInput")
    t_out = nc.dram_tensor("out", [128, p.tiles * HID], F32, kind="ExternalOutput")
    lo_ch = p.lo_chunks
    hi_ch = p.nchunks - p.lo_chunks
    h_lo = nc.dram_tensor("h_lo", [128, lo_ch * HID], BF16)
    h_hi = nc.dram_tensor("h_hi", [128, hi_ch * HID], BF16)

    SPAN = 64  # chunks per s1T span load

    # ---------------- context 1: stats + h ----------------
    tc1 = TileContext(nc)
    with tc1 as tc:
        with (tc.tile_pool(name="const", bufs=1) as cpool,
              tc.tile_pool(name="s1in", bufs=4) as apool,
              tc.tile_pool(name="span", bufs=2) as spool,
              tc.tile_pool(name="hout", bufs=3) as hpool,
              tc.tile_pool(name="psA", bufs=1, space="PSUM") as psA,
              tc.tile_pool(name="psZ", bufs=5, space="PSUM") as psZ,
              tc.tile_pool(name="small", bufs=1) as smpool):
            # constants
            w_f32 = []
            for r in range(2):
                w = cpool.tile([128, HID], F32, tag=f"wf{r}")
                nc.sync.dma_start(out=w[:], in_=t_wpre[r * 128:(r + 1) * 128, :])
                w_f32.append(w)
            gamma_sb = cpool.tile([1, HID], F32, tag="gm")
            nc.sync.dma_start(out=gamma_sb[:], in_=t_gamma[:])
            beta_sb = cpool.tile([1, HID], F32, tag="bt")
            nc.sync.dma_start(out=beta_sb[:], in_=t_beta[:])
            ones1_sb = cpool.tile([1, 128], F32, tag="on")
            nc.sync.dma_start(out=ones1_sb[:], in_=t_ones1[:])
            onesc_sb = cpool.tile([128, 1], F32, tag="onc")
            nc.sync.dma_start(out=onesc_sb[:], in_=t_onesc[:])
            wop_f32 = []
            for r in range(2):
                w = cpool.tile([128, HID], F32, tag=f"wo{r}")
                nc.sync.dma_start(out=w[:], in_=t_wop[r * 128:(r + 1) * 128, :])
                wop_f32.append(w)

            # ---- phase A: Gram ----
            gps = [psA.tile([128, C + 1], F32, tag=f"g{r}", name=f"gps{r}") for r in range(2)]
            ASPAN = 32
            naspans = math.ceil(p.nchunks / ASPAN)
            CW = C + 1
            def phaseA():
                for s in range(naspans):
                    j0 = s * ASPAN
                    j1 = min(p.nchunks, j0 + ASPAN)
                    s1t = apool.tile([128, ASPAN * CW], BF16, name="s1span")
                    nc.sync.dma_start(out=s1t[:, :(j1 - j0) * CW],
                                      in_=t_s1e[:, j0 * CW:j1 * CW])
                    for j in range(j0, j1):
                        co = (j - j0) * CW
                        for r in range(2):
                            nc.tensor.matmul(gps[r][:],
                                             lhsT=s1t[:, co + r * 128:co + r * 128 + 128],
                                             rhs=s1t[:, co:co + CW], start=(j == 0),
                                             stop=(j == p.nchunks - 1))
            if 'A' in phases:
                if nloop > 1:
                    with tc.For_i(0, nloop, 1):
                        phaseA()
                else:
                    phaseA()
            else:
                phaseA()  # once (stats needed)

            # ---- stats finalize (tiny, once) ----
            g_sb = []
            for r in range(2):
                g = smpool.tile([128, C + 1], F32, tag=f"gsb{r}")
                nc.vector.tensor_copy(out=g[:], in_=gps[r][:])
                g_sb.append(g)
            # B_mat = G @ W  (G symmetric; lhsT = G rows as [K,M])
            psB = psZ.tile([128, HID], F32, tag="zb")
            b_sb = []
            for r in range(2):
                for k in range(2):
                    nc.tensor.matmul(psB[:], lhsT=g_sb[k][:, r * 128:(r + 1) * 128],
                                     rhs=w_f32[k][:], start=(k == 0), stop=(k == 1))
                b = smpool.tile([128, HID], F32, tag=f"bsb{r}")
                nc.vector.tensor_copy(out=b[:], in_=psB[:])
                b_sb.append(b)

            def psum_colsum(tiles_in, tag):
                # sum over partitions of (tiles_in[0]+tiles_in[1]) -> [1, HID] in SBUF
                acc = smpool.tile([128, HID], F32, tag=tag)
                nc.vector.tensor_tensor(out=acc[:], in0=tiles_in[0][:],
                                        in1=tiles_in[1][:], op=mybir.AluOpType.add)
                ps = psA.tile([1, HID], F32, tag="zred", name=f"ps_{tag}")
                nc.tensor.matmul(ps[:], lhsT=onesc_sb[:], rhs=acc[:],
                                 start=True, stop=True)
                res = smpool.tile([1, HID], F32, tag=tag + "r", name=f"res_{tag}")
                nc.vector.tensor_copy(out=res[:], in_=ps[:])
                return res

            # mu_raw = sum_k s_k W[k, :]
            sw = []
            for r in range(2):
                t = smpool.tile([128, HID], F32, tag=f"sw{r}")
                nc.vector.tensor_scalar(out=t[:], in0=w_f32[r][:],
                                        scalar1=g_sb[r][:, C:C + 1], scalar2=None,
                                        op0=mybir.AluOpType.mult)
                sw.append(t)
            mu_acc = psum_colsum(sw, "mua")
            # d_raw = sum_k W[k,c] B[k,c]
            wb = []
            for r in range(2):
                t = smpool.tile([128, HID], F32, tag=f"wb{r}")
                nc.vector.tensor_tensor(out=t[:], in0=w_f32[r][:], in1=b_sb[r][:],
                                        op=mybir.AluOpType.mult)
                wb.append(t)
            d_acc = psum_colsum(wb, "da")
            invn = 1.0 / p.N
            mu = smpool.tile([1, HID], F32, tag="mu")
            nc.vector.tensor_scalar(out=mu[:], in0=mu_acc[:], scalar1=invn,
                                    scalar2=None, op0=mybir.AluOpType.mult)
            var = smpool.tile([1, HID], F32, tag="var")
            # var = d/N - mu^2
            nc.vector.tensor_scalar(out=var[:], in0=d_acc[:], scalar1=invn,
                                    scalar2=None, op0=mybir.AluOpType.mult)
            musq = smpool.tile([1, HID], F32, tag="musq")
            nc.vector.tensor_tensor(out=musq[:], in0=mu[:], in1=mu[:],
                                    op=mybir.AluOpType.mult)
            nc.vector.tensor_tensor(out=var[:], in0=var[:], in1=musq[:],
                                    op=mybir.AluOpType.subtract)
            nc.vector.tensor_scalar(out=var[:], in0=var[:], scalar1=BN_EPS,
                                    scalar2=None, op0=mybir.AluOpType.add)
            sq = smpool.tile([1, HID], F32, tag="sq")
            nc.scalar.activation(out=sq[:], in_=var[:],
                                 func=mybir.ActivationFunctionType.Sqrt,
                                 bias=0.0, scale=1.0)
            rs = smpool.tile([1, HID], F32, tag="rs")
            nc.vector.reciprocal(out=rs[:], in_=sq[:])
            a_vec = smpool.tile([1, HID], F32, tag="av")
            nc.vector.tensor_tensor(out=a_vec[:], in0=rs[:], in1=gamma_sb[:],
                                    op=mybir.AluOpType.mult)
            b_vec = smpool.tile([1, HID], F32, tag="bv")
            nc.vector.tensor_tensor(out=b_vec[:], in0=mu[:], in1=a_vec[:],
                                    op=mybir.AluOpType.mult)
            nc.vector.tensor_tensor(out=b_vec[:], in0=beta_sb[:], in1=b_vec[:],
                                    op=mybir.AluOpType.subtract)
            # broadcast A|B to 128 partitions via K=1 matmul
            ab_cat = smpool.tile([1, 2 * HID], F32, tag="abc")
            nc.vector.tensor_copy(out=ab_cat[:, :HID], in_=a_vec[:])
            nc.vector.tensor_copy(out=ab_cat[:, HID:], in_=b_vec[:])
            ps_ab = psZ.tile([128, 2 * HID], F32, tag="zb")
            nc.tensor.matmul(ps_ab[:], lhsT=ones1_sb[:], rhs=ab_cat[:],
                             start=True, stop=True)
            b_bc = cpool.tile([128, HID], F32, tag="bbc")
            nc.vector.tensor_copy(out=b_bc[:], in_=ps_ab[:, HID:])
            # scaled weights: Wsc = W * A (bf16); B row for K=1 add; Wop scaled
            wsc = []
            for r in range(2):
                w = cpool.tile([128, HID], BF16, tag=f"wsc{r}")
                nc.vector.tensor_tensor(out=w[:], in0=w_f32[r][:], in1=ps_ab[:, :HID],
                                        op=mybir.AluOpType.mult)
                wsc.append(w)
            b_row = cpool.tile([1, HID], BF16, tag="brow")
            nc.vector.tensor_copy(out=b_row[:], in_=b_vec[:])
            ones1_bf = cpool.tile([1, 128], BF16, tag="on16")
            nc.vector.tensor_copy(out=ones1_bf[:], in_=ones1_sb[:])
            wopsc = []
            for r in range(2):
                w = cpool.tile([128, HID], BF16, tag=f"wosc{r}")
                nc.vector.tensor_scalar(out=w[:], in0=wop_f32[r][:],
                                        scalar1=W_OP_SCALE, scalar2=None,
                                        op0=mybir.AluOpType.mult)
                wopsc.append(w)

            # ---- phase B: z = s1 @ Wsc (+B) -> relu -> h ----
            nspans = math.ceil(p.nchunks / SPAN)
            def phaseB():
                for s in range(nspans):
                    j0 = s * SPAN
                    j1 = min(p.nchunks, j0 + SPAN)
                    w_nodes = (j1 - j0) * 128
                    spans = []
                    for r in range(2):
                        sp = spool.tile([128, SPAN * 128], BF16, tag=f"sp{r}")
                        nc.sync.dma_start(
                            out=sp[:, :w_nodes],
                            in_=t_s1T[r * 128:(r + 1) * 128, j0 * 128:j0 * 128 + w_nodes])
                        spans.append(sp)
                    for j in range(j0, j1):
                        zc = psZ.tile([128, HID], F32, tag="zb")
                        coff = (j - j0) * 128
                        nc.tensor.matmul(zc[:], lhsT=spans[0][:, coff:coff + 128],
                                         rhs=wsc[0][:], start=True, stop=False)
                        nc.tensor.matmul(zc[:], lhsT=spans[1][:, coff:coff + 128],
                                         rhs=wsc[1][:], start=False, stop=True)
                        # h span buffering (chunk-major tables, 14-chunk spans)
                        HSPAN = 14
                        if j < p.lo_chunks:
                            tbl, jj, nch = h_lo, j, lo_ch
                        else:
                            tbl, jj, nch = h_hi, j - p.lo_chunks, hi_ch
                        hs = jj // HSPAN
                        ho = jj % HSPAN
                        he = min(nch, (hs + 1) * HSPAN) - hs * HSPAN
                        if ho == 0:
                            hsp = hpool.tile([128, HSPAN * HID], BF16, tag="hsp",
                                             name=f"hsp_{0 if tbl is h_lo else 1}_{hs % 3}")
                            phaseB.hsp = hsp
                        hsp = phaseB.hsp
                        zb = hpool.tile([128, HID], F32, tag="zb2")
                        nc.vector.tensor_tensor(out=zb[:], in0=zc[:], in1=b_bc[:],
                                                op=mybir.AluOpType.add)
                        if j % 2 == 0:
                            nc.vector.tensor_scalar(
                                out=hsp[:, ho * HID:(ho + 1) * HID], in0=zb[:],
                                scalar1=0.0, scalar2=None, op0=mybir.AluOpType.max)
                        else:
                            nc.scalar.activation(
                                out=hsp[:, ho * HID:(ho + 1) * HID], in_=zb[:],
                                func=mybir.ActivationFunctionType.Relu,
                                bias=0.0, scale=1.0)
                        if ho == he - 1:
                            nc.sync.dma_start(
                                out=tbl[:, hs * HSPAN * HID:(hs * HSPAN + he) * HID],
                                in_=hsp[:, :he * HID])
            if 'B' in phases:
                if nloop > 1:
                    with tc.For_i(0, nloop, 1):
                        phaseB()
                else:
                    phaseB()
            else:
                phaseB()  # once

        # ---------------- context 2: aggregate + output ----------------
        with (tc.tile_pool(name="c2", bufs=1) as cpool,
              tc.tile_pool(name="gat", bufs=10) as gpool,
              tc.tile_pool(name="sel", bufs=3) as selp,
              tc.tile_pool(name="epi", bufs=3) as epool,
              tc.tile_pool(name="psG", bufs=4, space="PSUM") as psG,
              tc.tile_pool(name="psT", bufs=2, space="PSUM") as psT,
              tc.tile_pool(name="psO", bufs=2, space="PSUM") as psO):
            idx_sb = cpool.tile([128, ktot * 8], I16, tag="idx")
            nc.sync.dma_start(out=idx_sb[:], in_=t_idx[:])
            drel_sb = cpool.tile([128, ktot], F32, tag="dr")
            nc.sync.dma_start(out=drel_sb[:], in_=t_drel[:])
            iota_sb = cpool.tile([128, 128], BF16, tag="io")
            nc.sync.dma_start(out=iota_sb[:], in_=t_iota[:])
            ident_sb = cpool.tile([128, 128], F32, tag="idn")
            nc.sync.dma_start(out=ident_sb[:], in_=t_ident[:])
            wop2 = []
            for r in range(2):
                w = cpool.tile([128, HID], F32, tag=f"wo2{r}")
                nc.sync.dma_start(out=w[:], in_=t_wop[r * 128:(r + 1) * 128, :])
                wb = cpool.tile([128, HID], BF16, tag=f"wo2b{r}")
                nc.vector.tensor_scalar(out=wb[:], in0=w[:], scalar1=W_OP_SCALE,
                                        scalar2=None, op0=mybir.AluOpType.mult)
                wop2.append(wb)

            qn = [0]
            def phaseC():
                for t in range(p.tiles):
                    gt = {}
                    st = {}
                    for hh in (0, 1) if 'g' in c_parts else ():
                        kk = int(K[t, hh])
                        g = gpool.tile([128, kk * HID], BF16, tag=f"g{hh}")
                        tbl = h_lo if hh == 0 else h_hi
                        o8 = int(off[t, hh]) * 8
                        nc.gpsimd.dma_gather(
                            out_ap=g[:].rearrange("p (c d) -> p c d", d=HID),
                            in_ap=tbl[:].rearrange("p (c d) -> (p c) d", d=HID),
                            idxs_ap=idx_sb[:, o8:o8 + kk * 8],
                            num_idxs=kk * 128, num_idxs_reg=kk * 128,
                            elem_size=HID, single_packet=False,
                            queue_num=qn[0] % 4)
                        qn[0] += 1
                        gt[hh] = g
                        ssp = selp.tile([128, kk * 128], BF16, tag=f"s{hh}")
                        o128 = int(off[t, hh]) * 128
                        nc.sync.dma_start(out=ssp[:], in_=t_S[:, o128:o128 + kk * 128])
                        st[hh] = ssp
                    agg = psG.tile([128, HID], F32, tag="agg")
                    nmm = int(K[t, 0] + K[t, 1])
                    ci = 0
                    if 's' not in c_parts or 'g' not in c_parts:
                        nc.vector.memset(agg[:], 0.0)
                    for hh in ((0, 1) if ('s' in c_parts and 'g' in c_parts) else ()):
                        kk = int(K[t, hh])
                        for c in range(kk):
                            nc.tensor.matmul(agg[:], lhsT=st[hh][:, c * 128:(c + 1) * 128],
                                             rhs=gt[hh][:, c * HID:(c + 1) * HID],
                                             start=(ci == 0), stop=(ci == nmm - 1))
                            ci += 1
                    # epilogue
                    if 'e' not in c_parts:
                        continue
                    ESPAN = 7
                    es = t // ESPAN
                    eo = t % ESPAN
                    e0 = es * ESPAN
                    e1 = min(p.tiles, e0 + ESPAN)
                    if eo == 0:
                        x0sp = epool.tile([128, ESPAN * HID], F32, tag="x0sp",
                                          name=f"x0sp{es % 2}")
                        nc.sync.dma_start(out=x0sp[:, :(e1 - e0) * HID],
                                          in_=t_x0[:, e0 * HID:e1 * HID])
                        outsp = epool.tile([128, ESPAN * HID], F32, tag="outsp",
                                           name=f"outsp{es % 2}")
                        phaseC.x0sp, phaseC.outsp = x0sp, outsp
                    x0sp, outsp = phaseC.x0sp, phaseC.outsp
                    sup = epool.tile([128, HID], F32, tag="sup")
                    # sup = 0.9*agg + 0.1*x0
                    nc.vector.tensor_scalar(out=sup[:], in0=agg[:],
                                            scalar1=(1.0 - ALPHA), scalar2=None,
                                            op0=mybir.AluOpType.mult)
                    x0sc = epool.tile([128, HID], F32, tag="x0sc")
                    nc.any.tensor_scalar(out=x0sc[:], in0=x0sp[:, eo * HID:(eo + 1) * HID],
                                         scalar1=ALPHA, scalar2=None,
                                         op0=mybir.AluOpType.mult)
                    nc.vector.tensor_tensor(out=sup[:], in0=sup[:], in1=x0sc[:],
                                            op=mybir.AluOpType.add)
                    # transpose sup -> supT (bf16)
                    trp = psT.tile([128, HID], F32, tag="tr")
                    for r in range(2):
                        nc.tensor.transpose(out=trp[:, r * 128:(r + 1) * 128],
                                            in_=sup[:, r * 128:(r + 1) * 128],
                                            identity=ident_sb[:])
                    supT = epool.tile([128, HID], BF16, tag="supT")
                    nc.any.tensor_copy(out=supT[:], in_=trp[:])
                    ops = psO.tile([128, HID], F32, tag="o")
                    nc.tensor.matmul(ops[:], lhsT=supT[:, :128], rhs=wop2[0][:],
                                     start=True, stop=False)
                    nc.tensor.matmul(ops[:], lhsT=supT[:, 128:], rhs=wop2[1][:],
                                     start=False, stop=True)
                    ut = epool.tile([128, HID], F32, tag="u")
                    nc.vector.tensor_tensor(out=ut[:], in0=sup[:], in1=ops[:],
                                            op=mybir.AluOpType.add)
                    nc.scalar.activation(out=outsp[:, eo * HID:(eo + 1) * HID],
                                         in_=ut[:],
                                         func=mybir.ActivationFunctionType.Relu,
                                         bias=0.0, scale=OUT_SCALE)
                    if t == e1 - 1:
                        nc.sync.dma_start(out=t_out[:, e0 * HID:e1 * HID],
                                          in_=outsp[:, :(e1 - e0) * HID])
            if 'C' in phases:
                if nloop > 1:
                    with tc.For_i(0, nloop, 1):
                        phaseC()
                else:
                    phaseC()

    nc.compile()
    return nc


def make_weight_inputs(prob, W_pre, gamma, beta_bn, W_op):
    return dict(
        wpre=np.asarray(W_pre, np.float32),
        gamma=np.asarray(gamma, np.float32).reshape(1, -1),
        beta=np.asarray(beta_bn, np.float32).reshape(1, -1),
        wop=np.asarray(W_op, np.float32),
    )


def unpack_out(prob, arr):
    """[128, tiles*HID] chunk-major -> [shard_pad, HID]"""
    return arr.reshape(128, prob.tiles, prob.HID).transpose(1, 0, 2).reshape(
        prob.shard_pad, prob.HID)


# ======================================================================
# Self-contained execution via PJRT (axon) and public kernel() entry
# ======================================================================
import jax
from jax.sharding import Mesh, PartitionSpec, NamedSharding
from jax.experimental.shard_map import shard_map
from concourse.bass2jax import _bass_exec_p, install_neuronx_cc_hook, partition_id_tensor


def _build_exec(nc, n_cores):
    install_neuronx_cc_hook()
    partition_name = nc.partition_id_tensor.name if nc.partition_id_tensor else None
    in_names, out_names, out_avals, zero_outs = [], [], [], []
    for alloc in nc.m.functions[0].allocations:
        if not isinstance(alloc, mybir.MemoryLocationSet):
            continue
        name = alloc.memorylocations[0].name
        if alloc.kind == "ExternalInput":
            if name != partition_name:
                in_names.append(name)
        elif alloc.kind == "ExternalOutput":
            shape = tuple(alloc.tensor_shape)
            dtype = mybir.dt.np(alloc.dtype)
            out_names.append(name)
            out_avals.append(jax.core.ShapedArray(shape, dtype))
            zero_outs.append(np.zeros(shape, dtype))
    n_params = len(in_names)
    n_outs = len(out_avals)
    all_in_names = list(in_names) + list(out_names)
    if partition_name is not None:
        all_in_names.append(partition_name)

    def _body(*args):
        operands = list(args)
        if partition_name is not None:
            operands.append(partition_id_tensor())
        outs = _bass_exec_p.bind(
            *operands, out_avals=tuple(out_avals), in_names=tuple(all_in_names),
            out_names=tuple(out_names), lowering_input_output_aliases=(),
            sim_require_finite=True, sim_require_nnan=True, nc=nc)
        return tuple(outs)

    devices = jax.devices()[:n_cores]
    mesh = Mesh(np.asarray(devices), ("core",))
    in_specs = (PartitionSpec("core"),) * (n_params + n_outs)
    out_specs = (PartitionSpec("core"),) * n_outs
    donate = tuple(range(n_params, n_params + n_outs))
    fn = jax.jit(shard_map(_body, mesh=mesh, in_specs=in_specs,
                           out_specs=out_specs, check_rep=False),
                 donate_argnums=donate, keep_unused=True)
    return dict(fn=fn, in_names=in_names, out_names=out_names,
                out_avals=out_avals, zero_outs=zero_outs, mesh=mesh,
                n_cores=n_cores)


def _place_inputs(ex, in_maps):
    sh = NamedSharding(ex["mesh"], PartitionSpec("core"))
    n_cores = ex["n_cores"]
    return [jax.device_put(
        np.concatenate([np.asarray(in_maps[c][name]) for c in range(n_cores)], axis=0), sh)
        for name in ex["in_names"]]


def _run(ex, dev_in):
    sh = NamedSharding(ex["mesh"], PartitionSpec("core"))
    n_cores = ex["n_cores"]
    zs = [jax.device_put(np.zeros((n_cores * z.shape[0], *z.shape[1:]), z.dtype), sh)
          for z in ex["zero_outs"]]
    outs = jax.block_until_ready(ex["fn"](*dev_in, *zs))
    return [
        {name: np.asarray(outs[i]).reshape(n_cores, *ex["out_avals"][i].shape)[c]
         for i, name in enumerate(ex["out_names"])}
        for c in range(n_cores)
    ]


_CACHE = {}


def _get_compiled(prob, meta, W_pre, gamma, beta_bn, W_op, key):
    if key not in _CACHE:
        nc = build_kernel(prob, meta, W_pre, gamma, beta_bn, W_op, nloop=1)
        _CACHE[key] = _build_exec(nc, prob.n_cores)
    return _CACHE[key]


def kernel(s0=None, s1=None, x_0=None, W_pre=None, gamma=None, beta_bn=None,
           W_op=None, edge_index=None, drop_prob=None, training=None, **_ignored):
    s1 = np.asarray(s1, np.float32)
    x_0 = np.asarray(x_0, np.float32)
    W_pre = np.asarray(W_pre, np.float32)
    gamma = np.asarray(gamma, np.float32)
    beta_bn = np.asarray(beta_bn, np.float32)
    W_op = np.asarray(W_op, np.float32)
    edge_index = np.asarray(edge_index)
    N, C = s1.shape
    HID = W_pre.shape[1]
    E = edge_index.shape[1]
    prob = Prob(N, E, C, HID, n_cores=8)
    in_maps, meta = host_prep(prob, s1, x_0, edge_index)
    key = (N, E, C, HID, int(np.int64(edge_index[:, ::97]).sum()), meta["ktot"])
    ex = _get_compiled(prob, meta, W_pre, gamma, beta_bn, W_op, key)
    wins = make_weight_inputs(prob, W_pre, gamma, beta_bn, W_op)
    full_maps = [{**m, **wins} for m in in_maps]
    dev_in = _place_inputs(ex, full_maps)
    res = _run(ex, dev_in)
    out = np.concatenate(
        [unpack_out(prob, res[m]["out"])[:prob.shard] for m in range(prob.n_cores)],
        axis=0)
    return np.ascontiguousarray(out[:N]).astype(np.float32)

